# revision 73
# baseline (speedup 1.0000x reference)
"""Trainium2 Bass kernel for nn_CapsuleLayer (dynamic routing).

Math (per independent column c=(k,b,o), vector p = pred[k,b,:,o] of length N):
    logits stay proportional to p:  logits_t = p * V_t  with scalar V_t.
    iter 1: c uniform -> s1 = mean_n(p);  v1 = squash(s1); V1 = v1
    iter t: Z = sum_n exp(V*p), Y = sum_n p*exp(V*p), s = Y/Z,
            v = squash(s) = s*|s|/(1+s^2), V += v
    output = v from the last iteration.

Sharding: data-parallel over batch (32 of 256 per core, 8 cores).

Engine plan (ACT is the bottleneck and runs mostly exp):
  ACT: 80 exp tiles [128,1152] fp16, per-partition scale=V, fused
       accum_out=Z.  1145ns + 187ns accum read each => ~106.6us floor.
       Plus 2 merged psA evacs per bg (scalar.copy) to relieve DVE.
  Products y=p*e: POOLSET tiles (0,1,2,3,8) via Pool tensor_tensor
       (2381ns; GPSIMD has no accum port and only add/mult ALU ops),
       their Y accums via DVE tensor_scalar 4x (360ns) at slots where
       the y is already finished; the rest are DVE tensor_tensor (2x,
       660ns) + tensor_scalar accum (4x, 360ns) pairs.
  PSUM->SBUF evacs: merged [128,1024] tiles (one evac per 8 ngs), on
       DVE except the 2/bg on ACT; warmup bg0 round-robins DVE/ACT.
       GPSIMD cannot touch PSUM (birverifier rejects it).
  t1 smalls on DVE (|s| via max(s,-s); abs_max is not a walrus ALU op),
       a2 smalls on Pool emitted after p8 so nothing queues behind them.

Exp stream per (bg,it): e0 e1 p0 e2 p1A0 e3 p2 e4 p3A1 e5 p4A2 e6 p5
  e7 p6 e8 p7 A3 S_t1 e9 p8 p9 A8 S_a2  -- products lag exps by one
  unit and accums sit at their ready times, so no engine head-blocks.
  gen(bg+1) (9 merged-evac units) interleaves from route unit 16 on, so
  the it2 V-chain smalls are never queued behind next-bg evacs; the
  serial-DMA ramp orders wr+xsal before any xbd chunk.
"""

import sys

sys.path.insert(0, "/opt/trn_rl_repo")

from contextlib import ExitStack

import numpy as np

import concourse.bass as bass  # noqa: F401
import concourse.bacc as bacc
import concourse.tile as tile
from concourse import mybir
from concourse.bass_utils import run_bass_kernel_spmd

# ---- problem constants (hardcoded per harness contract) ----
B, N, CIN = 256, 1152, 8
K, O = 10, 16
KO = K * O            # 160
NCORES = 8
BSH = B // NCORES     # 32 batch per core
BG, BL = 4, 8         # batch groups x lanes (BSH = BG*BL)
NG, NL = 72, 16       # n-groups x n-lanes (N = NG*NL)
NQ = NG // 4          # 18
F32 = mybir.dt.float32
F16 = mybir.dt.float16

_cache = {}


# ----------------------------------------------------------------------------
# host-side input prep
# ----------------------------------------------------------------------------
def _prep_shared(w):
    # Wr[ng, 8*nl+i, 16*k+o] = w[k, 16*ng+nl, i, o]; ship partition-major
    wr = np.transpose(
        w.reshape(K, NG, NL, CIN, O), (1, 2, 3, 0, 4)
    ).reshape(NG, 128, KO).astype(np.float16)
    wr = np.ascontiguousarray(np.transpose(wr, (1, 0, 2)).reshape(128, NG * KO))
    ident32 = np.eye(32, dtype=np.float32)
    # sel2[ko2, (bl4,ko2')] = (ko2==ko2')  -> broadcast ko2-rows to 4 bl4 slots
    sel2 = np.tile(np.eye(32, dtype=np.float32), (1, 4))      # [32, 128]
    return wr, ident32, sel2


def _prep_core_inputs(x, w):
    wr, ident32, sel2 = _prep_shared(w)
    in_maps = []
    for c in range(NCORES):
        xc = x[c * BSH:(c + 1) * BSH]                          # [32, N, CIN]
        # xs[ng, 8*nl+i, b] = xc[b, 16*ng+nl, i]
        xs = np.transpose(
            xc.reshape(BSH, NG, NL, CIN), (1, 2, 3, 0)
        ).reshape(NG, 128, BSH).astype(np.float16)
        xs = np.ascontiguousarray(
            np.transpose(xs, (1, 0, 2)).reshape(128, NG * BSH))
        # xbd[bg, ng, (nl',i), (nl,bl)] = xc[8bg+bl, 16ng+nl, i] * (nl==nl')
        xbd = np.zeros((BG, NG, NL, CIN, NL, BL), dtype=np.float16)
        xs5 = np.transpose(
            xc.reshape(BG, BL, NG, NL, CIN), (0, 2, 3, 4, 1)
        ).astype(np.float16)                                   # [bg,ng,nl,i,bl]
        for r in range(NL):
            xbd[:, :, r, :, r, :] = xs5[:, :, r, :, :]
        xbd = np.ascontiguousarray(
            np.transpose(xbd.reshape(BG, NG, 128, 128),
                         (0, 2, 1, 3)).reshape(BG, 128, NG * 128))
        in_maps.append({
            "xbd": xbd, "xs": xs, "wr": wr,
            "sel2": sel2, "ident32": ident32,
        })
    return in_maps


# ----------------------------------------------------------------------------
# device program
# ----------------------------------------------------------------------------
def _interleave(*gens):
    gens = list(gens)
    while gens:
        nxt = []
        for g in gens:
            try:
                next(g)
                nxt.append(g)
            except StopIteration:
                pass
        gens = nxt


def _build_program(T):
    nc = bacc.Bacc("TRN2", target_bir_lowering=False, debug=False,
                   enable_asserts=False)

    xbd_d = nc.dram_tensor("xbd", [BG, 128, NG * 128], F16, kind="ExternalInput").ap()
    xs_d = nc.dram_tensor("xs", [128, NG * BSH], F16, kind="ExternalInput").ap()
    wr_d = nc.dram_tensor("wr", [128, NG * KO], F16, kind="ExternalInput").ap()
    sel2_d = nc.dram_tensor("sel2", [32, 128], F32, kind="ExternalInput").ap()
    id_d = nc.dram_tensor("ident32", [32, 32], F32, kind="ExternalInput").ap()
    out1_d = nc.dram_tensor("out1", [BG, 128, BL], F32, kind="ExternalOutput").ap()
    # out2[(bl4,ko2), (bg,gr)] -> v[8+kk, 8bg+4gr+bl4, o], ko2 = 16kk+o
    out2_d = nc.dram_tensor("out2", [128, 2 * BG], F32, kind="ExternalOutput").ap()

    mult = mybir.AluOpType.mult
    add = mybir.AluOpType.add
    absmax = mybir.AluOpType.abs_max
    EXP = mybir.ActivationFunctionType.Exp

    WCH = 4           # wr DMA chunks (18 ngs each)
    XCH = 3           # xbd DMA chunks per bg (24 ngs each)
    NGC = NG // XCH   # 24
    NGT = NG // 3     # 24 ngs per a2 psum third
    # product tiles on Pool (tensor_tensor): the first four t1 tiles form a
    # serial Pool chain that finishes before the t1 smalls need their accums,
    # plus a2 tile 8 after the smalls.  Everything else is DVE tt+ts pairs.
    POOLSET = (0, 1, 2, 3, 8)
    ACC_AT = {4: 0, 6: 1, 7: 2}   # exp-slot -> Pool tile whose accum to emit

    with tile.TileContext(nc) as tc, ExitStack() as ctx:
        consts = ctx.enter_context(tc.tile_pool(name="consts", bufs=1))
        a1p = ctx.enter_context(tc.tile_pool(name="a1", bufs=3))
        a2p = ctx.enter_context(tc.tile_pool(name="a2n", bufs=4))
        xbdp = ctx.enter_context(tc.tile_pool(name="xbd", bufs=5))
        ep = ctx.enter_context(tc.tile_pool(name="e", bufs=16))
        yp = ctx.enter_context(tc.tile_pool(name="y", bufs=12))
        scrp = ctx.enter_context(tc.tile_pool(name="scr", bufs=2))
        smp = ctx.enter_context(tc.tile_pool(name="sm", bufs=4))
        # 4 banks psA (2-deep [128,1024] merged tiles: one evac per 8 ngs)
        # + 2 psB + m1's two banks = 8
        psA = ctx.enter_context(tc.tile_pool(name="psA", bufs=2, space="PSUM"))
        psB = ctx.enter_context(tc.tile_pool(name="psB", bufs=2, space="PSUM"))
        psM = ctx.enter_context(tc.tile_pool(name="psM", bufs=1, space="PSUM"))
        psT = ctx.enter_context(tc.tile_pool(name="psT", bufs=1, space="PSUM"))

        # ---- resident inputs ----
        wrs = consts.tile([128, NG * KO], F16, tag="wrs", name="wrs")
        xsal = consts.tile([128, NG * BSH], F16, tag="xsal", name="xsal")
        sel2s = consts.tile([32, 128], F32, tag="sel2", name="sel2")
        id32 = consts.tile([32, 32], F32, tag="id32", name="id32")
        # exp bias -ln(1024)-ish: e' = exp(V*p - 6.93); cancels in s = Y/Z and
        # keeps the fp16 y = p*e' product finite (max |p*e| ~ 2.4e5 otherwise)
        bexp = consts.tile([128, 1], F32, tag="bexp", name="bexp")
        nc.gpsimd.memset(bexp[:], -6.93)

        st = {}
        a1t_of = {}
        a2n_of = {}
        warm_tgl = [0]

        def evac_warm(dst, src, late=False):
            # warmup-phase evac: DVE/ACT round-robin (ACT is idle before the
            # first routing window; GPSIMD cannot access PSUM per birverifier)
            if warm_tgl[0] % 2 == 0 or late:
                nc.vector.tensor_copy(dst, src)
            else:
                nc.scalar.copy(dst, src)
            warm_tgl[0] += 1

        def squash(s_ap, P, W, tg, ve=None):
            """v = s*|s|/(1+s*s) as a fresh [P, W] f32 tile.  Tensor ops on
            `ve` (DVE or Pool); reciprocal is DVE-only hardware."""
            ve = ve or nc.vector
            n2 = smp.tile([P, W], F32, tag=f"sq_n2_{tg}", name=f"sq_n2_{tg}")
            ve.tensor_tensor(n2[:], s_ap, s_ap, mult)
            d = smp.tile([P, W], F32, tag=f"sq_d_{tg}", name=f"sq_d_{tg}")
            ve.tensor_scalar_add(d[:], n2[:], 1.0)
            r = smp.tile([P, W], F32, tag=f"sq_r_{tg}", name=f"sq_r_{tg}")
            nc.vector.reciprocal(r[:], d[:])
            ns = smp.tile([P, W], F32, tag=f"sq_ns_{tg}", name=f"sq_ns_{tg}")
            ve.tensor_scalar_mul(ns[:], s_ap, -1.0)
            # max is not a legal GPSIMD ALU op; always run |s| on DVE
            a = smp.tile([P, W], F32, tag=f"sq_a_{tg}", name=f"sq_a_{tg}")
            nc.vector.tensor_tensor(a[:], s_ap, ns[:], mybir.AluOpType.max)
            t = smp.tile([P, W], F32, tag=f"sq_t_{tg}", name=f"sq_t_{tg}")
            ve.tensor_tensor(t[:], s_ap, a[:], mult)
            v = smp.tile([P, W], F32, tag=f"sq_v_{tg}", name=f"sq_v_{tg}")
            ve.tensor_tensor(v[:], t[:], r[:], mult)
            return v

        # ------------------------------------------------------------------
        def input_dmas():
            # wr + xsal first: m1 and gen0 both need the full weights, and
            # the modeled DMA device is serial -- anything queued between wr
            # chunks delays V1 and the whole ramp
            g = NG // WCH
            nc.sync.dma_start(wrs[:, 0:g * KO], wr_d[:, 0:g * KO])
            nc.sync.dma_start(xsal[:], xs_d)
            for chv in range(1, WCH):
                g0, g1 = chv * g, (chv + 1) * g
                nc.sync.dma_start(wrs[:, g0 * KO:g1 * KO],
                                  wr_d[:, g0 * KO:g1 * KO])
            nc.sync.dma_start(sel2s[:], sel2_d)
            nc.sync.dma_start(id32[:], id_d)

        def xbd_dma(bg):
            # bg 0/1 go on the sync queue, strictly ORDERED BEHIND the wr
            # chunks (the modeled DMA device is serial, so an unordered xbd
            # transfer would steal device slots from the ramp-critical wr);
            # later bgs use the ACT queue's hardware DGE (device idle by then)
            eng = nc.sync if bg < 2 else nc.scalar
            tiles = []
            for chx in range(XCH):
                xt = xbdp.tile([128, NGC * 128], F16, tag="xbd", name="xbd")
                eng.dma_start(
                    xt[:], xbd_d[bg, :, chx * NGC * 128:(chx + 1) * NGC * 128])
                tiles.append(xt)
            st[("xbt", bg)] = tiles

        # ------------------------------------------------------------------
        def m1_phase():
            """iter-1: n-summed matmul -> V1 for t1 [128,32] and a2 [128,8]."""
            m1ps = psM.tile([32, KO], F32, tag="m1ps", name="m1ps")
            for ng in range(NG):
                nc.tensor.matmul(
                    m1ps[:],
                    xsal[:, ng * BSH:(ng + 1) * BSH],
                    wrs[:, ng * KO:(ng + 1) * KO],
                    start=(ng == 0), stop=(ng == NG - 1))
            m1s = smp.tile([32, KO], F32, tag="m1s", name="m1s")
            nc.vector.tensor_copy(m1s[:], m1ps[:])
            t1 = psT.tile([128, 32], F32, tag="psT", name="psT")
            nc.tensor.transpose(t1[:], m1s[:, 0:128], id32[:])
            s1 = smp.tile([128, 32], F32, tag="s1t1", name="s1t1")
            nc.vector.tensor_scalar_mul(s1[:], t1[:], 1.0 / N)
            v1 = squash(s1[:], 128, 32, "t1v1")
            st["V_t1_1"] = v1              # [128(ko), 32(bg,bl)]
            yield
            # a2: s1a [32(ko2), 32(b)] -> broadcast to [128(bl4,ko2), 32(b)]
            t2 = psT.tile([128, 32], F32, tag="psT", name="psT")
            nc.tensor.transpose(t2[:32, :], m1s[:, 128:KO], id32[:])
            s1a = smp.tile([32, 32], F32, tag="s1a2", name="s1a2")
            nc.vector.tensor_scalar_mul(s1a[:], t2[:32, :], 1.0 / N)
            vb = psT.tile([128, 32], F32, tag="psT", name="psT")
            nc.tensor.matmul(vb[:], sel2s[:], s1a[:], start=True, stop=True)
            s1f = smp.tile([128, 32], F32, tag="s1a2f", name="s1a2f")
            nc.vector.tensor_copy(s1f[:], vb[:])
            v1a = squash(s1f[:], 128, 32, "a2v1")  # [128(bl4,ko2), 32(bg,bl)]
            # Va2_1[(bl4,ko2), (bg,gr)] = v1a[(bl4,ko2), 8bg+4gr+bl4]
            va = smp.tile([128, 2 * BG], F32, tag="Va2_1", name="Va2_1")
            for bl4 in range(4):
                nc.vector.tensor_copy(
                    va[32 * bl4:32 * bl4 + 32, :],
                    v1a[32 * bl4:32 * bl4 + 32, bl4::4])
            st["Va2_1"] = va
            yield

        # ------------------------------------------------------------------
        def gen_phase(bg, warm=False):
            a1t = a1p.tile([128, BL * N], F16, tag="a1", name="a1")
            a1t_of[bg] = a1t
            a1v = a1t[:].rearrange("p (b g l) -> p g l b", b=BL, g=NG, l=NL)
            a2n = [a2p.tile([128, N], F16, tag="a2n", name="a2n")
                   for _ in range(2)]
            a2n_of[bg] = a2n
            xbt = st[("xbt", bg)]
            pb = [None, None]
            pb_n0 = 0
            NQH = NG // 8                     # 9 blocks of 8 ng
            for Qh in range(NQH):
                pa = psA.tile([128, 1024], F32, tag="psA", name="psA")
                if Qh % 3 == 0:
                    # new third tiles for a2: [128, 384] per gr
                    pb = [psB.tile([128, NGT * NL], F32, tag="psB", name="psB")
                          for _ in range(2)]
                    pb_n0 = Qh * 8 * NL
                for j in range(8):
                    ng = 8 * Qh + j
                    xt = xbt[ng // NGC]
                    nb = (ng % NGC) * 128
                    w0 = wrs[:, ng * KO:ng * KO + 128]
                    w1 = wrs[:, ng * KO + 128:ng * KO + KO]
                    rhs = xt[:, nb:nb + 128]
                    nc.tensor.matmul(pa[:, j * 128:(j + 1) * 128],
                                     w0, rhs, start=True, stop=True)
                    c0 = (ng % NGT) * NL
                    for gr in range(2):
                        for bl4 in range(4):
                            rhs16 = xt[:, nb + 4 * gr + bl4:nb + 128:8]
                            nc.tensor.matmul(
                                pb[gr][32 * bl4:32 * bl4 + 32, c0:c0 + NL],
                                w1, rhs16, start=True, stop=True,
                                tile_position=(0, 32 * bl4))
                # evacuate psA -> A1[bg]; dst/src iteration order = (g, l, b)
                dst = a1v[:, 8 * Qh:8 * Qh + 8, :, :]
                src = pa[:].rearrange("p (g l b) -> p g l b",
                                      g=8, l=NL, b=BL)
                if warm:
                    evac_warm(dst, src, late=(Qh >= 7))
                elif Qh in (7, 8):
                    # two psA evacs per bg on ACT: DVE is the tighter budget;
                    # the last two Qhs so their matmuls are done when the ACT
                    # queue reaches them late in the route window
                    nc.scalar.copy(dst, src)
                else:
                    nc.vector.tensor_copy(dst, src)
                if Qh % 3 == 2:
                    for gr in range(2):
                        dst = a2n[gr][:, pb_n0:pb_n0 + NGT * NL]
                        if warm:
                            evac_warm(dst, pb[gr][:])
                        else:
                            nc.vector.tensor_copy(dst, pb[gr][:])
                yield

        # ------------------------------------------------------------------
        def route(bg):
            """Unit stream per iteration (driver interleaves gen between
            units).  Tiles u=0..7 are t1 per-bl, u=8,9 are a2 per-gr.
            Products: u < NPOOL -> Pool stt (no accum port on Pool), with
            the Y accum done by a DVE tensor_scalar (4x) two units later;
            u >= NPOOL -> DVE tt (2x) + ts accum (4x) back-to-back.
            t1 smalls run mostly on Pool under the two a2 exps."""
            a1t = a1t_of[bg]
            a2n = a2n_of[bg]
            vl_t1 = None
            vl_a2 = None

            for it in range(2, T + 1):
                if it == 2:
                    Vt1, vo1 = st["V_t1_1"], 8 * bg
                    Va2, vo2 = st["Va2_1"], 2 * bg
                else:
                    Vt1, vo1 = st[("V_t1", bg, it - 1)], 0
                    Va2, vo2 = st[("Va2", bg, it - 1)], 0
                Z = smp.tile([128, BL], F32, tag="Zt1", name="Zt1")
                Y = smp.tile([128, BL], F32, tag="Yt1", name="Yt1")
                Za = smp.tile([128, 2], F32, tag="Za2", name="Za2")
                Ya = smp.tile([128, 2], F32, tag="Ya2", name="Ya2")
                es = [None] * 10
                ys = [None] * 10

                def p_slice(u):
                    if u < 8:
                        return a1t[:, u * N:(u + 1) * N]
                    return a2n[u - 8][:]

                def emit_exp(u):
                    e = ep.tile([128, N], F16, tag="e", name="e")
                    es[u] = e
                    if u < 8:
                        sc, zc = Vt1[:, vo1 + u:vo1 + u + 1], Z[:, u:u + 1]
                    else:
                        g = u - 8
                        sc, zc = Va2[:, vo2 + g:vo2 + g + 1], Za[:, g:g + 1]
                    nc.scalar.activation(e[:], p_slice(u), EXP, bias=bexp[:],
                                         scale=sc, accum_out=zc)

                def emit_prod(u):
                    yc = Y[:, u:u + 1] if u < 8 else Ya[:, u - 8:u - 7]
                    if u == 8 and it == T and bg == BG - 1:
                        # final window: keep tile 8 off the slow Pool chain
                        emit_prod_dve(u)
                        return
                    if u in POOLSET:
                        y = yp.tile([128, N], F16, tag="y", name="y")
                        ys[u] = y
                        nc.gpsimd.tensor_tensor(y[:], p_slice(u), es[u][:],
                                                mult)
                    else:
                        y = yp.tile([128, N], F16, tag="y", name="y")
                        nc.vector.tensor_tensor(y[:], p_slice(u), es[u][:],
                                                mult)
                        scr = scrp.tile([128, N], F16, tag="scr", name="scr")
                        nc.vector.tensor_scalar(
                            out=scr[:], in0=y[:], scalar1=1.0, scalar2=0.0,
                            op0=mult, op1=add, accum_out=yc)

                def emit_prod_dve(u):
                    # force the DVE tt+ts pair regardless of POOLSET
                    yc = Y[:, u:u + 1] if u < 8 else Ya[:, u - 8:u - 7]
                    y = yp.tile([128, N], F16, tag="y", name="y")
                    nc.vector.tensor_tensor(y[:], p_slice(u), es[u][:], mult)
                    scr = scrp.tile([128, N], F16, tag="scr", name="scr")
                    nc.vector.tensor_scalar(
                        out=scr[:], in0=y[:], scalar1=1.0, scalar2=0.0,
                        op0=mult, op1=add, accum_out=yc)

                def emit_accum(u):
                    # DVE 4x accum of a Pool-produced y into its Y slot
                    yc = Y[:, u:u + 1] if u < 8 else Ya[:, u - 8:u - 7]
                    scr = scrp.tile([128, N], F16, tag="scr", name="scr")
                    nc.vector.tensor_scalar(
                        out=scr[:], in0=ys[u][:], scalar1=1.0, scalar2=0.0,
                        op0=mult, op1=add, accum_out=yc)

                def smalls_t1():
                    nonlocal vl_t1
                    r = smp.tile([128, BL], F32, tag="rt1", name="rt1")
                    nc.vector.reciprocal(r[:], Z[:])
                    s = smp.tile([128, BL], F32, tag="st1", name="st1")
                    nc.vector.tensor_tensor(s[:], Y[:], r[:], mult)
                    v = squash(s[:], 128, BL, "t1")
                    Vn = smp.tile([128, BL], F32, tag="Vt1n", name="Vt1n")
                    nc.vector.tensor_tensor(
                        Vn[:], Vt1[:, vo1:vo1 + BL], v[:], add)
                    st[("V_t1", bg, it)] = Vn
                    vl_t1 = v

                def smalls_a2():
                    # on Pool AFTER p8 (last Pool op of the window): Pool is
                    # idle then and nothing queues behind it in this window.
                    # Final window: all-DVE, the Pool roundtrip would sit on
                    # the kernel-exit critical path.
                    nonlocal vl_a2
                    ve = nc.vector if (it == T and bg == BG - 1) else nc.gpsimd
                    r = smp.tile([128, 2], F32, tag="ra2", name="ra2")
                    nc.vector.reciprocal(r[:], Za[:])
                    s = smp.tile([128, 2], F32, tag="sa2", name="sa2")
                    ve.tensor_tensor(s[:], Ya[:], r[:], mult)
                    v = squash(s[:], 128, 2, "a2", ve=ve)
                    Vn = smp.tile([128, 2], F32, tag="Va2n", name="Va2n")
                    ve.tensor_tensor(
                        Vn[:], Va2[:, vo2:vo2 + 2], v[:], add)
                    st[("Va2", bg, it)] = Vn
                    vl_a2 = v

                if it == T and bg == BG - 1:
                    # final window: a2 exps early (their V is ready), so the
                    # a2 products + smalls + out2 DMA finish mid-window and
                    # only the t1 chain trails the last exp
                    emit_exp(0)
                    yield
                    emit_exp(1)
                    yield
                    emit_prod(0)
                    yield
                    emit_exp(8)
                    yield
                    emit_prod(1)
                    yield
                    emit_exp(9)
                    yield
                    emit_prod(8)
                    yield
                    emit_exp(2)
                    yield
                    emit_prod(9)
                    yield
                    smalls_a2()
                    yield
                    emit_exp(3)
                    yield
                    emit_prod(2)
                    yield
                    emit_exp(4)
                    yield
                    emit_prod(3)
                    emit_accum(0)
                    yield
                    emit_exp(5)
                    yield
                    emit_accum(1)
                    yield
                    emit_exp(6)
                    yield
                    emit_prod(4)
                    emit_accum(2)
                    yield
                    emit_exp(7)
                    yield
                    emit_prod(5)
                    emit_accum(3)
                    yield
                    emit_prod(6)
                    yield
                    emit_prod(7)
                    yield
                    smalls_t1()
                    yield
                else:
                    # unit stream: products lag exps by one unit; Pool-tile
                    # accums are placed at slots where their y is already done
                    emit_exp(0)
                    yield
                    for u in range(1, 9):
                        emit_exp(u)
                        yield
                        emit_prod(u - 1)
                        if u in ACC_AT:
                            emit_accum(ACC_AT[u])
                        yield
                    # drain: last pool accum, t1 smalls (all deps ready)
                    emit_accum(3)
                    yield
                    smalls_t1()
                    yield
                    emit_exp(9)
                    yield
                    emit_prod(8)
                    yield
                    emit_prod(9)
                    emit_accum(8)
                    smalls_a2()
                    yield

            if T == 1:
                v1_t1 = st["V_t1_1"][:, 8 * bg:8 * bg + 8]
                v1_a2 = st["Va2_1"][:, 2 * bg:2 * bg + 2]
            else:
                v1_t1 = vl_t1[:]
                v1_a2 = vl_a2[:]
            nc.sync.dma_start(out1_d[bg], v1_t1)
            nc.sync.dma_start(out2_d[:, 2 * bg:2 * bg + 2], v1_a2)
            yield

        def seq(*gens):
            for g in gens:
                yield from g

        def slow(g, k=2, skip=0):
            # advance g once every k interleave rounds, after `skip` rounds
            cnt = 0
            while True:
                cnt += 1
                if cnt > skip and (cnt - skip) % k == 0:
                    try:
                        next(g)
                    except StopIteration:
                        return
                yield

        input_dmas()
        xbd_dma(0)
        xbd_dma(1)
        _interleave(seq(m1_phase(), gen_phase(0, warm=True)))
        # gen(bg)'s 9 units are spread over ~18 route units (both its of the
        # window) so the DVE evac load per iteration stays under its slack
        for bg in range(1, BG):
            if bg + 1 < BG:
                xbd_dma(bg + 1)
            _interleave(route(bg - 1), slow(gen_phase(bg), 1, skip=16))
        _interleave(route(BG - 1))

    nc.compile()
    return nc


def _get_program(T):
    if T not in _cache:
        _cache[T] = _build_program(T)
    return _cache[T]


# ----------------------------------------------------------------------------
# host-side output assembly
# ----------------------------------------------------------------------------
def _assemble(results):
    v = np.zeros((K, B, 1, 1, O), dtype=np.float32)
    for c, res in enumerate(results):
        o1 = res["out1"]          # [BG, 128=(16k+o), BL]
        o2 = res["out2"]          # [128=(32bl4+16kk+o), 8=(2bg+gr)]
        b0 = c * BSH
        # o1[bg, 16k+o, bl] -> v[k, b0+8bg+bl, 0, 0, o]
        t = o1.reshape(BG, 8, O, BL).transpose(1, 0, 3, 2)  # [k, bg, bl, o]
        v[:8, b0:b0 + BSH, 0, 0, :] = t.reshape(8, BSH, O)
        # o2[(bl4, kk, o), (bg, gr)] -> v[8+kk, b0+8bg+4gr+bl4, 0, 0, o]
        t2 = o2.reshape(4, 2, O, BG, 2)                     # bl4,kk,o,bg,gr
        for bl4 in range(4):
            for bg in range(BG):
                for gr in range(2):
                    v[8:, b0 + 8 * bg + 4 * gr + bl4, 0, 0, :] = \
                        t2[bl4, :, :, bg, gr]
    return v


def run(x, routing_weights, num_iterations, trace=False):
    T = int(num_iterations)
    x = np.asarray(x, dtype=np.float32)
    w = np.asarray(routing_weights, dtype=np.float32)
    nc = _get_program(T)
    in_maps = _prep_core_inputs(x, w)
    kw = {}
    if trace:
        kw = dict(trace=True, trace_cores=list(range(NCORES)))
    res = run_bass_kernel_spmd(nc, in_maps, core_ids=list(range(NCORES)), **kw)
    return _assemble(res.results), res


def kernel(x, routing_weights, num_iterations):
    out, _ = run(x, routing_weights, num_iterations)
    return out
